# revision 12
# baseline (speedup 1.0000x reference)
"""FFM (field-aware factorization machine) forward pass on 8 Trainium2 cores.

Math (per sample b):
    linear[b] = X[b,:] @ w1 + b0
    C[i,j]    = sum_k v[i, field[j], k] * v[j, field[i], k]   (pair coefficients)
    inter[b]  = sum_{i<j} C[i,j] X[b,i] X[b,j]
    out[b]    = sigmoid(linear[b] + inter[b])

Strategy (v4):
  * Precompute Cm = strict-upper(C) on host, scale by S=4096 so its ~6e-4-scale
    entries sit in fp8-e4m3 normal range; fold w1*S into row 511 (structurally
    zero in the strict upper triangle) and feed the stationary operand a 1.0 in
    the matching row, so Y = X@Cm + 1*w1^T*S and the epilogue row-sum yields
    S*(inter + linear) at zero extra cost.  Sigmoid applies scale=1/S.
  * inter[b] = rowsum((X @ Cm) * X): the heavy op is the [B,512]x[512,512]
    matmul, data-parallel over batch across 8 cores (4096 rows/core).
  * Everything ships fp8-e4m3 (4.25MB/core vs 8.5 bf16); matmuls run in
    DoubleRow perf mode: contraction 256 per pass, 2 passes per 128-row tile.
    Both operands' (k, r) -> feature maps agree: feature = pass*256 + r*128 + p.
  * PE waits are hoisted one tile ahead so the next tile's LDWEIGHTS lands in
    the shadow of the current matmul via the PE reorder window (a blocking
    wait between tiles otherwise exposes the ~213ns DoubleRow weight load
    per tile; steady-state is 2x216ns per tile with the hoist).
  * Epilogue on THREE engines: VectorE does 20 tiles' (Y*X)+rowsum straight
    from PSUM (1x STT, ~625ns); ScalarE drains the other 12 tiles' PSUM to
    SBUF bf16 (freeing the bank early) and GpSimdE does those products.
    DVE has no 2x uop for scalar_tensor_tensor, so offload beats dtype games.
  * A dummy activation preloads the Copy table off the critical path; the
    sigmoids run 16/12/4-wide at the end so the Sigmoid table load hides
    behind waiting for the last tiles; one fence + a single 128-descriptor
    output DMA.
  * DMA queues: sync carries X^T then late X-natural, gpsimd carries C +
    early X-natural, scalar only bias + y (a big dma_start costs ~2.9us/MB
    of issuing-sequencer time, which would delay the drain loop).

Raw bass (no TileContext: this container's walrus rejects Tile's multi-wait
encodings and the TENSOR_TENSOR_REDUCE direct-ISA opcode).
"""

import contextlib

import numpy as np
import ml_dtypes

P = 128          # partitions / tile rows
F = 512          # features
NPASS = 2        # DoubleRow contraction passes (256 features each)
NCORES = 8
B = 32768
BSH = B // NCORES   # 4096 rows per core
NT = BSH // P       # 32 batch tiles per core
NPSUM = 8           # psum bank rotation depth (warmup bank recycled)
NWARM = 8           # dummy warm-up matmuls bridging the first DMA arrivals
NYBUF = 6           # drained-Y sbuf slots
CSCALE = 4096.0     # fp8 scale on Cm (and the folded w1 row)

FP8 = ml_dtypes.float8_e4m3fn

# epilogue: all tiles through VectorE's fused 1x psum STT (DVE has no 2x
# uop for scalar_tensor_tensor, and walrus rejects compute ops on GpSimd)
DRAIN = []
DIRECT = list(range(NT))
RANKS = {t: i for i, t in enumerate(DRAIN)}
RANKV = {t: i for i, t in enumerate(DIRECT)}


def _groups(singles, pairs_until, quad):
    gs = [(t, 1) for t in range(singles)]
    t = singles
    while t < pairs_until:
        gs.append((t, 2))
        t += 2
    while t < NT:
        n = min(quad, NT - t)
        gs.append((t, n))
        t += n
    return gs


XT_G = _groups(4, 8, 4)
XN_G = _groups(2, 8, 4)
XN_SPLIT = 6    # first groups (tiles 0..13) on the gpsimd queue, rest on sync


def _build_bass():
    import concourse.bass as bass
    from concourse import mybir

    nc = bass.Bass()

    # [p, t, j]: xnat[p, t, j] = X8[t*128 + p, j]
    xnat = nc.declare_dram_parameter("xnat", [P, NT, F], mybir.dt.float8e4, isOutput=False)[:]
    # [p, t, pass, r, b]: xt[p, t, s, r, b] = X8[t*128 + b, s*256 + r*128 + p]
    xt = nc.declare_dram_parameter("xt", [P, NT, NPASS, 2, P], mybir.dt.float8e4, isOutput=False)[:]
    # [p, pass, r, j]: cmat[p, s, r, j] = Cm_s[s*256 + r*128 + p, j]
    cmat = nc.declare_dram_parameter("cmat", [P, NPASS, 2, F], mybir.dt.float8e4, isOutput=False)[:]
    bias = nc.declare_dram_parameter("bias", [1], mybir.dt.float32, isOutput=False)[:]
    y = nc.declare_dram_parameter("y", [P, NT], mybir.dt.float32, isOutput=True)[:]

    xt_of = {}
    for gi, (t0, n) in enumerate(XT_G):
        for t in range(t0, t0 + n):
            xt_of[t] = gi
    xn_of = {}
    for gi, (t0, n) in enumerate(XN_G):
        for t in range(t0, t0 + n):
            xn_of[t] = gi

    with contextlib.ExitStack() as st:
        ec = st.enter_context
        c_sb = ec(nc.sbuf_tensor([P, NPASS, 2, F], mybir.dt.float8e4))
        b_sb = ec(nc.sbuf_tensor([P, 1], mybir.dt.float32))
        xbuf = ec(nc.sbuf_tensor([P, NT, F], mybir.dt.float8e4))
        xtbuf = ec(nc.sbuf_tensor([P, NT, NPASS, 2, P], mybir.dt.float8e4))
        ybuf = ec(nc.sbuf_tensor([P, NYBUF, F], mybir.dt.bfloat16))
        dump = ec(nc.sbuf_tensor([P, F], mybir.dt.bfloat16))
        dump2 = ec(nc.sbuf_tensor([P, F], mybir.dt.bfloat16))
        acc = ec(nc.sbuf_tensor([P, NT], mybir.dt.float32))
        out_sb = ec(nc.sbuf_tensor([P, NT], mybir.dt.float32))
        ps = [ec(nc.psum_tensor(f"ps{i}", [P, F], mybir.dt.float32)) for i in range(NPSUM)]
        ps_warm = ps[NPSUM - 1]   # warmups use the last bank; tile 7 reuses it


        s_c = ec(nc.semaphore(name="s_c"))
        s_b = ec(nc.semaphore(name="s_b"))
        s_xt = [ec(nc.semaphore(name=f"s_xt{i}")) for i in range(len(XT_G))]
        s_xn = [ec(nc.semaphore(name=f"s_xn{i}")) for i in range(len(XN_G))]
        s_mm = ec(nc.semaphore(name="s_mm"))
        s_dr = ec(nc.semaphore(name="s_dr"))
        s_mul = ec(nc.semaphore(name="s_mul"))     # VectorE direct-tile STTs
        s_gp = ec(nc.semaphore(name="s_gp"))       # GpSimd drained-tile STTs
        s_act = ec(nc.semaphore(name="s_act"))
        s_out = ec(nc.semaphore(name="s_out"))

        block = ec(nc.Block())

        def xn_issue(eng, gi):
            t0n, nn = XN_G[gi]
            eng.dma_start(
                out=xbuf[:, t0n : t0n + nn], in_=xnat[:, t0n : t0n + nn]
            ).then_inc(s_xn[gi], 16)

        def acc_ready_waits(eng, T):
            # all acc[:, 0:T] written: direct tiles via s_mul, drained via s_gp
            nv = sum(1 for t in DIRECT if t < T)
            ng = sum(1 for t in DRAIN if t < T)
            if nv:
                eng.wait_ge(s_mul, nv)
            if ng:
                eng.wait_ge(s_gp, ng)

        @block.sync
        def _(sync):
            # sync HWDGE queue: C then X^T groups (the PE's diet)
            sync.dma_start(out=c_sb[:], in_=cmat[:]).then_inc(s_c, 16)
            for gi in range(len(XT_G)):
                t0, n = XT_G[gi]
                sync.dma_start(
                    out=xtbuf[:, t0 : t0 + n], in_=xt[:, t0 : t0 + n]
                ).then_inc(s_xt[gi], 16)

        @block.scalar
        def _(scalar):
            # scalar queue carries only bias + y: a large dma_start costs
            # ~2.9us/MB of sequencer time and would delay the drain loop.
            scalar.dma_start(out=b_sb[:], in_=bias.to_broadcast([P, 1])).then_inc(s_b, 16)
            for gi in range(len(XN_G)):
                xn_issue(scalar, gi)
            for i, t in enumerate(DRAIN):
                scalar.wait_ge(s_mm, t + 1)
                if i >= NYBUF:
                    # ybuf slot reuse: gpsimd product i-NYBUF must be done
                    scalar.wait_ge(s_gp, i - NYBUF + 1)
                nc.scalar.activation(
                    out=ybuf[:, i % NYBUF, :],
                    in_=ps[t % NPSUM][:],
                    func=mybir.ActivationFunctionType.Copy,
                ).then_inc(s_dr, 1)
            scalar.wait_ge(s_b, 16)
            for g in range(NT // 4):
                # all 4 accs of the batch ready (s_mul counts STTs in order)
                scalar.wait_ge(s_mul, 4 * g + 4)
                nc.scalar.activation(
                    out=out_sb[:, 4 * g : 4 * g + 4],
                    in_=acc[:, 4 * g : 4 * g + 4],
                    func=mybir.ActivationFunctionType.Sigmoid,
                    bias=b_sb[:],
                    scale=1.0 / CSCALE,
                ).then_inc(s_act, 1)
                # fence: the output dma must not outrun the activation's drain
                scalar.wait_ge(s_act, g + 1)
                scalar.dma_start(
                    out=y[:, 4 * g : 4 * g + 4], in_=out_sb[:, 4 * g : 4 * g + 4]
                ).then_inc(s_out, 16)
            scalar.wait_ge(s_out, 16 * (NT // 4))

        @block.tensor
        def _(tensor):
            for _w in range(NWARM):
                nc.tensor.matmul(
                    ps_warm[:],
                    xtbuf[:, 0, 0, :, :],
                    c_sb[:, 0, :, :],
                    start=True,
                    stop=True,
                    perf_mode=mybir.MatmulPerfMode.DoubleRow,
                )
            # waits for tile 0 (later tiles' waits are hoisted one tile early
            # so the next LDWEIGHTS overlaps the current matmul)
            tensor.wait_ge(s_c, 16)
            for t in range(NT):
                gi = xt_of[t]
                if t == XT_G[gi][0]:
                    tensor.wait_ge(s_xt[gi], 16)
                if t >= NPSUM:
                    # psum bank reuse: epilogue(t-NPSUM) must be done reading
                    tensor.wait_ge(s_mul, t - NPSUM + 1)
                pst = ps[t % NPSUM]
                for s in range(NPASS):
                    mm = nc.tensor.matmul(
                        pst[:],
                        xtbuf[:, t, s, :, :],
                        c_sb[:, s, :, :],
                        start=(s == 0),
                        stop=(s == NPASS - 1),
                        perf_mode=mybir.MatmulPerfMode.DoubleRow,
                        skip_group_check=True,
                    )
                mm.then_inc(s_mm, 1)

        @block.vector
        def _(vector):
            seen = set()
            for t in DIRECT:
                gi = xn_of[t]
                if gi not in seen:
                    seen.add(gi)
                    vector.wait_ge(s_xn[gi], 16)
                vector.wait_ge(s_mm, t + 1)
                nc.vector.scalar_tensor_tensor(
                    out=dump[:],
                    in0=ps[t % NPSUM][:],
                    scalar=0.0,
                    in1=xbuf[:, t, :],
                    op0=mybir.AluOpType.add,
                    op1=mybir.AluOpType.mult,
                    accum_out=acc[:, t : t + 1],
                ).then_inc(s_mul, 1)


    return nc


def _host_prep(X, w1, b, v, feature2field):
    """Returns per-core input maps."""
    X = np.asarray(X, dtype=np.float32)
    w1 = np.asarray(w1, dtype=np.float32)
    b = np.asarray(b, dtype=np.float32)
    v = np.asarray(v, dtype=np.float32)
    f2f = np.asarray(feature2field, dtype=np.int32)

    # Pair-coefficient matrix: C[i,j] = sum_k v[i, f2f[j], k] * v[j, f2f[i], k]
    A = v[:, f2f, :]                      # [n, n, k]
    C = (A * A.transpose(1, 0, 2)).sum(axis=2)
    Cm = np.triu(C, 1)
    # Fold the linear term: row F-1 of strict-upper Cm is all zeros.
    Cm[F - 1, :] = w1[:, 0]
    c8 = (Cm * CSCALE).astype(FP8)
    # [p, pass, r, j] with row index i = s*256 + r*128 + p
    c_host = np.ascontiguousarray(c8.reshape(NPASS, 2, P, F).transpose(2, 0, 1, 3))

    X8 = X.astype(FP8)
    in_maps = []
    for c in range(NCORES):
        Xc8 = X8[c * BSH : (c + 1) * BSH]             # [4096, 512]
        # xnat layout [p, t, j] = Xc[t*128 + p, j]
        xnat = np.ascontiguousarray(Xc8.reshape(NT, P, F).transpose(1, 0, 2))
        # xt layout [p, t, s, r, b] = Xc[t*128 + b, s*256 + r*128 + p]
        xtl = np.ascontiguousarray(
            Xc8.reshape(NT, P, NPASS, 2, P).transpose(4, 0, 2, 3, 1)
        )
        # stationary row for feature 511 (s=1, r=1, p=127) := 1.0 (w1 fold)
        xtl[P - 1, :, NPASS - 1, 1, :] = FP8(1.0)
        in_maps.append({"xnat": xnat, "xt": xtl, "cmat": c_host, "bias": b})
    return in_maps


def _run(in_maps, trace=False):
    from concourse.bass_utils import run_bass_kernel_spmd

    nc = _build_bass()
    res = run_bass_kernel_spmd(nc, in_maps, core_ids=list(range(NCORES)), trace=trace)
    out = np.concatenate([r["y"].reshape(P, NT).T.reshape(-1) for r in res.results])
    return out, res


def kernel(X, w1, b, v, feature2field):
    in_maps = _host_prep(X, w1, b, v, feature2field)
    out, _ = _run(in_maps, trace=False)
    return out.astype(np.float32)


if __name__ == "__main__":
    pass
